# revision 17
# baseline (speedup 1.0000x reference)
"""BiaffineAttn Trainium2 kernel.

Math (per batch b):
    t    = x2 @ U                      [S, D]
    attn = t @ x1^T + (x1 @ bias)[None, :]
    p    = softmax(attn, axis=-1)
    out  = relu((p @ x1) @ fc_w^T + fc_b)    [S, F]

Sharding: data-parallel over batch B=8, one batch per NeuronCore.

Two algebraic restructures vs the naive form:
  * (p @ x1) @ fc_w^T = p @ (x1 @ fc_w^T): a one-time prepass
    x1fc = x1 @ fc_w^T [S, F] (bf16) replaces the per-superblock
    S*S*D + S*D*F output matmuls with S*S*F — 40% fewer output-side MACs.
  * (x2 @ U) @ x1^T = x2 @ (U @ x1^T): a one-time prepass
    y1T = U @ x1^T [D, S] (fp32r) absorbs the whole x2-side projection into
    startup — same FLOPs, but the steady loop loses the per-superblock tT
    matmul phase and its PSUM/copy traffic entirely, which keeps the tensor
    engine in long uninterrupted matmul runs (the HAM clock gate throttles
    the PE array to half clock after ~3.4us of low activity).

Per-core pipeline (attention in TRANSPOSED orientation so the softmax key
dimension t' lands on SBUF partitions; N=512 moving chunks):
    scoresT = attn^T [t', s]        fp32r y1T 128x128 stationaries, fp32r x2T
    pT      = exp(scoresT - rowmax_bcast + kb)  (kb = x1 @ bias folded into
                                    exp's per-partition bias operand)
    rowsum  = ones_col^T @ pT       PE K-accumulated 1-row matmul (no DVE
                                    reduce pipeline)
    outT    = relu(recip * (x1fc^T @ pT) + fcb)   stationary x1fc tiles
  rowmax: elementwise-max chase over the 16 t'-tiles on VectorE, then a
  128-partition reduce via PE transposes + ones-matmul broadcast.

The score path stays fp32r (fp22) end-to-end: with scores ~N(0, 32^2) the
softmax is highly peaked and any bf16 rounding upstream (~0.2 absolute on
scores) turns into ~20% errors on near-tie attention weights, blowing the
max-abs error metric.  bf16 is confined to the post-exp path (p, x1fc),
where errors average out over the 2048-key contraction.

SBUF packing (the y1T residency costs 8MB): the U^T tiles borrow the score
ring (they die in the prepass, scores first exist after it), x1T streaming
chunks and all x2T chunks share one 24-slot ring whose reuse order matches
first-use order, and mrow/srow/rrow share one ring.

Engine placement: PSUM->SBUF score copies on ScalarE, the (scores - max)
subtract on GpSimdE, max-chase/reduces/normalize on VectorE, so no engine
queue ever gates the PE at a phase boundary.

Host side: transposes x1/x2/U/fc_w per-core (fp32 DMA transpose does not
exist on TRN2) and transposes the [F,S] per-core output back to [S,F].
"""

import os
import sys
from contextlib import ExitStack

import numpy as np

for _p in ("/opt/trn_rl_repo", os.path.expanduser("~/.axon_site/_ro/trn_rl_repo")):
    if os.path.isdir(_p) and _p not in sys.path:
        sys.path.insert(0, _p)

import concourse.bass as bass
import concourse.mybir as mybir
import concourse.tile as tile
from concourse import bacc

B = 8
S = 2048          # sequence length (both s and t')
D = 1024          # d_model
F = 512           # fc output dim
P = 128
SB = 512          # s superblock (moving free dim of every matmul)
NSB = S // SB     # 4
DC = D // P       # 8 contraction chunks of d / e
TC = S // P       # 16 t' tiles
FPG = F // P      # 4 output row-tiles
FP32 = mybir.dt.float32
FP32R = mybir.dt.float32r
BF16 = mybir.dt.bfloat16
AF = mybir.ActivationFunctionType
ALU = mybir.AluOpType
AX = mybir.AxisListType


def build_nc():
    nc = bacc.Bacc(
        "TRN2",
        target_bir_lowering=False,
        debug=False,
        enable_asserts=False,
    )

    x1t_d = nc.dram_tensor("x1t", [D, S], FP32R, kind="ExternalInput")
    x2t_d = nc.dram_tensor("x2t", [D, S], FP32R, kind="ExternalInput")
    ut_d = nc.dram_tensor("ut", [D, D], FP32R, kind="ExternalInput")
    fcwt_d = nc.dram_tensor("fcwt", [D, F], FP32R, kind="ExternalInput")
    bias_d = nc.dram_tensor("bias", [D, 1], FP32R, kind="ExternalInput")
    fcb_d = nc.dram_tensor("fcb", [F, 1], FP32, kind="ExternalInput")
    outt_d = nc.dram_tensor("outt", [F, S], FP32, kind="ExternalOutput")

    with tile.TileContext(nc) as tc, ExitStack() as ctx:
        # ---------- pools ----------
        p_y1t = ctx.enter_context(tc.tile_pool(name="y1ts", bufs=NSB * DC))
        p_stream = ctx.enter_context(tc.tile_pool(name="stream", bufs=24))
        p_sc = ctx.enter_context(tc.tile_pool(name="scores", bufs=TC))
        p_x1fc = ctx.enter_context(tc.tile_pool(name="x1fcs", bufs=TC))
        p_fcw = ctx.enter_context(tc.tile_pool(name="fcwres", bufs=DC))
        p_kb = ctx.enter_context(tc.tile_pool(name="kbcols", bufs=TC))
        p_bc = ctx.enter_context(tc.tile_pool(name="biascols", bufs=DC))
        p_fcb = ctx.enter_context(tc.tile_pool(name="fcbcols", bufs=FPG))
        p_ones = ctx.enter_context(tc.tile_pool(name="ones", bufs=1))
        p_pb = ctx.enter_context(tc.tile_pool(name="pbf", bufs=6))
        p_aux = ctx.enter_context(tc.tile_pool(name="aux", bufs=1))
        p_row = ctx.enter_context(tc.tile_pool(name="rows", bufs=1))
        p_mcol = ctx.enter_context(tc.tile_pool(name="mcols", bufs=SB // P))
        p_out = ctx.enter_context(tc.tile_pool(name="outs", bufs=2))
        # PSUM: 4 banks for the output accumulators, 1 for the rowsum, 3
        # general-purpose (MM2/transposes/broadcasts/prepass) = 8 banks.
        p_pso = ctx.enter_context(tc.tile_pool(name="pso", bufs=FPG, space="PSUM"))
        p_pssum = ctx.enter_context(tc.tile_pool(name="pssum", bufs=1, space="PSUM"))
        p_psg = ctx.enter_context(tc.tile_pool(name="psg", bufs=3, space="PSUM"))

        # ---------- constants (no DMA dependency) ----------
        identity = p_ones.tile([P, P], FP32, name="ident", tag="ident")
        nc.gpsimd.memset(identity[:], 0.0)
        nc.gpsimd.affine_select(
            out=identity[:], in_=identity[:], compare_op=ALU.not_equal,
            fill=1.0, base=0, pattern=[[-1, P]], channel_multiplier=1,
        )
        ones_row = p_ones.tile([1, P], FP32R, name="ones_row", tag="ones_row")
        nc.scalar.activation(ones_row[:], identity[0:1, :], AF.Identity, bias=1.0, scale=0.0)
        ones_col = p_ones.tile([P, 1], BF16, name="ones_col", tag="ones_col")
        nc.scalar.activation(ones_col[:], identity[:, 0:1], AF.Identity, bias=1.0, scale=0.0)

        # ---------- input DMAs ----------
        # Emission order is DMA queue order AND stream-ring slot order: the
        # ring reuses slots in first-use order (x1T tg chunks die tg-by-tg in
        # the prepass, freeing slots for later chunks / x2T superblocks).
        bias_cols = []
        for i in range(DC):
            b_t = p_bc.tile([P, 1], FP32R, name=f"bc{i}", tag="bc")
            nc.sync.dma_start(b_t[:], bias_d[i * P : (i + 1) * P, :])
            bias_cols.append(b_t)
        chunks = [[None] * DC for _ in range(NSB)]

        def load_chunks(tg):
            # split each chunk across two DMA queues (partition halves) —
            # a single 256KB transfer is queue-latency-bound at startup
            for ec in range(DC):
                c = p_stream.tile([P, SB], FP32R, name=f"x1tc_{tg}_{ec}", tag="st")
                h = P // 2
                for hf in range(2):
                    nc.sync.dma_start(
                        c[hf * h : (hf + 1) * h, :],
                        x1t_d[
                            ec * P + hf * h : ec * P + (hf + 1) * h,
                            tg * SB : (tg + 1) * SB,
                        ],
                    )
                chunks[tg][ec] = c

        def emit_x2t(sb):
            s0 = sb * SB
            x2t_tiles = []
            for dc in range(DC):
                x2_t = p_stream.tile([P, SB], FP32R, name=f"x2t_{sb}_{dc}", tag="st")
                nc.sync.dma_start(x2_t[:], x2t_d[dc * P : (dc + 1) * P, s0 : s0 + SB])
                x2t_tiles.append(x2_t)
            return x2t_tiles

        load_chunks(0)
        fcw_tiles = []
        for i in range(DC):
            f_t = p_fcw.tile([P, F], FP32R, name=f"fcw{i}", tag="fcw")
            nc.sync.dma_start(f_t[:], fcwt_d[i * P : (i + 1) * P, :])
            fcw_tiles.append(f_t)
        # U^T tiles borrow the score ring: they die in the prepass, before
        # the first score tile exists.
        ut_tiles = [[None, None] for _ in range(DC)]
        for ec in range(DC):
            for h in range(2):
                t = p_sc.tile([P, SB], FP32R, name=f"ut{ec}_{h}", tag="sc")
                nc.sync.dma_start(
                    t[:], ut_d[ec * P : (ec + 1) * P, h * SB : (h + 1) * SB]
                )
                ut_tiles[ec][h] = t
        load_chunks(1)
        load_chunks(2)
        load_chunks(3)
        x2t_cur = emit_x2t(0)
        fcb_cols = []
        for i in range(FPG):
            c_t = p_fcb.tile([P, 1], FP32, name=f"fcb{i}", tag="fcb")
            nc.sync.dma_start(c_t[:], fcb_d[i * P : (i + 1) * P, :])
            fcb_cols.append(c_t)

        # ---------- prepass: kb = x1 @ bias, x1fc = x1 @ fc_w^T,
        #                     y1T = U @ x1^T ----------
        kb_cols = [None] * TC
        x1fc_tiles = [None] * TC
        y1t_tiles = [[None] * NSB for _ in range(DC)]
        for tg in range(NSB):
            ps_kb = p_pssum.tile([1, SB], FP32, name=f"pskb{tg}", tag="pssum")
            for ec in range(DC):
                nc.tensor.matmul(
                    ps_kb[:], bias_cols[ec][:], chunks[tg][ec][:],
                    start=(ec == 0), stop=(ec == DC - 1),
                )
            kb_r = p_row.tile([1, SB], FP32R, name=f"kbrow{tg}", tag="kbrow")
            nc.vector.tensor_copy(kb_r[:], ps_kb[:])
            for sub in range(4):
                ti = tg * 4 + sub
                ps_x = p_psg.tile([P, F], FP32, name=f"psx{ti}", tag="psg")
                for ec in range(DC):
                    nc.tensor.matmul(
                        ps_x[:],
                        chunks[tg][ec][:, sub * P : (sub + 1) * P],
                        fcw_tiles[ec][:],
                        start=(ec == 0), stop=(ec == DC - 1),
                    )
                xf = p_x1fc.tile([P, F], BF16, name=f"x1fc{ti}", tag="x1fc")
                nc.vector.tensor_copy(xf[:], ps_x[:])
                x1fc_tiles[ti] = xf
                ps_c = p_psg.tile([P, 4], FP32, name=f"pskc{ti}", tag="psg")
                nc.tensor.matmul(
                    ps_c[:], kb_r[0:1, sub * P : (sub + 1) * P],
                    ones_row[0:1, 0:4], start=True, stop=True,
                )
                kb_c = p_kb.tile([P, 1], FP32, name=f"kb{ti}", tag="kb")
                nc.vector.tensor_copy(kb_c[:], ps_c[:, 0:1])
                kb_cols[ti] = kb_c
            for dc in range(DC):
                ps_y = p_psg.tile([P, SB], FP32, name=f"psy{tg}_{dc}", tag="psg")
                for ec in range(DC):
                    nc.tensor.matmul(
                        ps_y[:],
                        ut_tiles[ec][dc // 4][:, (dc % 4) * P : (dc % 4 + 1) * P],
                        chunks[tg][ec][:],
                        start=(ec == 0), stop=(ec == DC - 1),
                    )
                y_t = p_y1t.tile([P, SB], FP32R, name=f"y1t{dc}_{tg}", tag="y1t")
                nc.vector.tensor_copy(y_t[:], ps_y[:])
                y1t_tiles[dc][tg] = y_t

        # ---------- steady loop ----------
        # MM2 tile emitter; the first two tiles of each superblock are
        # emitted during the previous superblock's tail so the PE has work
        # while VectorE runs the recip chain.
        sc_all = [[None] * TC for _ in range(NSB)]
        maxaccs = [None] * NSB

        def mm2_tile(sb, ti, x2t_tiles):
            tg, sub = ti // 4, ti % 4
            ps_s = p_psg.tile([P, SB], FP32, name=f"pss{sb}_{ti}", tag="psg")
            for dc in range(DC):
                nc.tensor.matmul(
                    ps_s[:],
                    y1t_tiles[dc][tg][:, sub * P : (sub + 1) * P],
                    x2t_tiles[dc][:],
                    start=(dc == 0), stop=(dc == DC - 1),
                )
            s_t = p_sc.tile([P, SB], FP32, name=f"sc{sb}_{ti}", tag="sc")
            nc.scalar.copy(s_t[:], ps_s[:])
            if ti == 0:
                maxaccs[sb] = p_aux.tile(
                    [P, SB], FP32, name=f"maxacc{sb}", tag="maxacc"
                )
                nc.vector.tensor_copy(maxaccs[sb][:], ps_s[:])
            else:
                nc.vector.tensor_max(maxaccs[sb][:], maxaccs[sb][:], ps_s[:])
            sc_all[sb][ti] = s_t

        for sb in range(NSB):
            s0 = sb * SB
            x2t_tiles = x2t_cur
            if sb + 1 < NSB:
                x2t_cur = emit_x2t(sb + 1)

            # ---- MM2: scoresT tiles + running elementwise max ----
            for ti in range(0 if sb == 0 else 2, TC):
                mm2_tile(sb, ti, x2t_tiles)
            sc_tiles = sc_all[sb]
            maxacc = maxaccs[sb]

            # ---- per-s max over partitions: transpose + free reduce ----
            mrow = p_row.tile([1, SB], FP32R, name=f"mrow{sb}", tag="rowsc")
            mcols = []
            for blk in range(SB // P):
                ps_tr = p_psg.tile([P, P], FP32, name=f"ptr{sb}_{blk}", tag="psg")
                nc.tensor.transpose(
                    ps_tr[:], maxacc[:, blk * P : (blk + 1) * P], identity[:]
                )
                mcol = p_mcol.tile([P, 1], FP32, name=f"mcol{sb}_{blk}", tag="mcol")
                nc.vector.reduce_max(mcol[:], ps_tr[:], axis=AX.X)
                mcols.append(mcol)
            for blk in range(SB // P):
                ps_rr = p_psg.tile([1, P], FP32, name=f"prr{sb}_{blk}", tag="psg")
                nc.tensor.transpose(ps_rr[:], mcols[blk][:], identity[:])
                nc.vector.tensor_copy(mrow[:, blk * P : (blk + 1) * P], ps_rr[:])
            ps_mb = p_psg.tile([P, SB], FP32, name=f"pmb{sb}", tag="psg")
            nc.tensor.matmul(ps_mb[:], ones_row[:], mrow[:], start=True, stop=True)

            # ---- exp phase, chased by the PE: rowsum + output matmuls ----
            # the (scores - max) subtract reads the broadcast PSUM bank
            # directly on VectorE — no SBUF maxb staging copy.
            ps_sum = p_pssum.tile([1, SB], FP32, name=f"pssm{sb}", tag="pssum")
            ps_o = [
                p_pso.tile([P, SB], FP32, name=f"pso{sb}_{ft}", tag="pso")
                for ft in range(FPG)
            ]
            for ti in range(TC):
                nc.vector.tensor_sub(sc_tiles[ti][:], sc_tiles[ti][:], ps_mb[:])
                p_t = p_pb.tile([P, SB], BF16, name=f"pb{sb}_{ti}", tag="pb")
                nc.scalar.activation(
                    p_t[:], sc_tiles[ti][:], AF.Exp, bias=kb_cols[ti][:], scale=1.0
                )
                nc.tensor.matmul(
                    ps_sum[:], ones_col[:], p_t[:],
                    start=(ti == 0), stop=(ti == TC - 1),
                )
                for ft in range(FPG):
                    nc.tensor.matmul(
                        ps_o[ft][:],
                        x1fc_tiles[ti][:, ft * P : (ft + 1) * P],
                        p_t[:],
                        start=(ti == 0), stop=(ti == TC - 1),
                    )

            # next superblock's first MM2 tiles backfill the recip handoff
            if sb + 1 < NSB:
                for ti in range(2):
                    mm2_tile(sb + 1, ti, x2t_cur)

            # ---- recip + broadcast ----
            srow = p_row.tile([1, SB], FP32, name=f"srow{sb}", tag="rowsc")
            nc.vector.tensor_copy(srow[:], ps_sum[:])
            rrow = p_row.tile([1, SB], FP32R, name=f"rrow{sb}", tag="kbrow")
            with nc.allow_low_precision(reason="recip feeds fp32r matmul; fp22 ok"):
                nc.vector.reciprocal(rrow[:], srow[:])
            ps_rb = p_psg.tile([P, SB], FP32, name=f"prb{sb}", tag="psg")
            nc.tensor.matmul(ps_rb[:], ones_row[:], rrow[:], start=True, stop=True)
            recipb = p_aux.tile([P, SB], FP32, name=f"recipb{sb}", tag="recipb")
            nc.vector.tensor_copy(recipb[:], ps_rb[:])

            # ---- normalize + bias + relu + store ----
            for ft in range(FPG):
                tmp = p_out.tile([P, SB], FP32, name=f"tmp{sb}_{ft}", tag="tmp")
                nc.vector.tensor_mul(tmp[:], ps_o[ft][:], recipb[:])
                o_out = p_out.tile([P, SB], FP32, name=f"oo{sb}_{ft}", tag="oo")
                nc.scalar.activation(
                    o_out[:], tmp[:], AF.Relu, bias=fcb_cols[ft][:], scale=1.0
                )
                nc.sync.dma_start(outt_d[ft * P : (ft + 1) * P, s0 : s0 + SB], o_out[:])

    nc.compile()
    return nc


_NC_CACHE = None


def _get_nc():
    global _NC_CACHE
    if _NC_CACHE is None:
        _NC_CACHE = build_nc()
    return _NC_CACHE


def make_in_maps(x1, x2, U, bias, fc_w, fc_b):
    x1 = np.ascontiguousarray(np.asarray(x1, dtype=np.float32))
    x2 = np.ascontiguousarray(np.asarray(x2, dtype=np.float32))
    U = np.asarray(U, dtype=np.float32)
    bias = np.asarray(bias, dtype=np.float32).reshape(D, 1)
    fc_w = np.asarray(fc_w, dtype=np.float32)
    fc_b = np.asarray(fc_b, dtype=np.float32).reshape(F, 1)
    ut = np.ascontiguousarray(U.T)
    fcwt = np.ascontiguousarray(fc_w.T)
    in_maps = []
    for b in range(B):
        in_maps.append(
            {
                "x1t": np.ascontiguousarray(x1[b].T),
                "x2t": np.ascontiguousarray(x2[b].T),
                "ut": ut,
                "fcwt": fcwt,
                "bias": bias,
                "fcb": fc_b,
            }
        )
    return in_maps


def kernel(x1, x2, U, bias, fc_w, fc_b):
    from concourse.bass_utils import run_bass_kernel_spmd

    nc = _get_nc()
    in_maps = make_in_maps(x1, x2, U, bias, fc_w, fc_b)
    res = run_bass_kernel_spmd(nc, in_maps, core_ids=list(range(B)))
    out = np.stack([np.ascontiguousarray(r["outt"].T) for r in res.results])
    return out.astype(np.float32)


# revision 19
# speedup vs baseline: 1.1650x; 1.1650x over previous
"""BiaffineAttn Trainium2 kernel.

Math (per batch b):
    t    = x2 @ U                      [S, D]
    attn = t @ x1^T + (x1 @ bias)[None, :]
    p    = softmax(attn, axis=-1)
    out  = relu((p @ x1) @ fc_w^T + fc_b)    [S, F]

Sharding: data-parallel over batch B=8, one batch per NeuronCore.

Two algebraic restructures vs the naive form:
  * (p @ x1) @ fc_w^T = p @ (x1 @ fc_w^T): a one-time prepass
    x1fc = x1 @ fc_w^T [S, F] (bf16) replaces the per-superblock
    S*S*D + S*D*F output matmuls with S*S*F — 40% fewer output-side MACs.
  * (x2 @ U) @ x1^T = x2 @ (U @ x1^T): a one-time prepass
    y1T = U @ x1^T [D, S] (fp32r) absorbs the whole x2-side projection into
    startup — same FLOPs, but the steady loop loses the per-superblock tT
    matmul phase and its PSUM/copy traffic entirely, which keeps the tensor
    engine in long uninterrupted matmul runs (the HAM clock gate throttles
    the PE array to half clock after ~3.4us of low activity).

Per-core pipeline (attention in TRANSPOSED orientation so the softmax key
dimension t' lands on SBUF partitions; N=512 moving chunks):
    scoresT = attn^T [t', s]        fp32r y1T 128x128 stationaries, fp32r x2T
    pT      = exp(scoresT - rowmax_bcast + kb)  (kb = x1 @ bias folded into
                                    exp's per-partition bias operand)
    rowsum  = ones_col^T @ pT       PE K-accumulated 1-row matmul (no DVE
                                    reduce pipeline)
    outT    = relu(recip * (x1fc^T @ pT) + fcb)   stationary x1fc tiles
  rowmax: elementwise-max chase over the 16 t'-tiles on VectorE, then a
  128-partition reduce via PE transposes + ones-matmul broadcast.

The score path stays fp32r (fp22) end-to-end: with scores ~N(0, 32^2) the
softmax is highly peaked and any bf16 rounding upstream (~0.2 absolute on
scores) turns into ~20% errors on near-tie attention weights, blowing the
max-abs error metric.  bf16 is confined to the post-exp path (p, x1fc),
where errors average out over the 2048-key contraction.

SBUF packing (the y1T residency costs 8MB): the U^T tiles borrow the score
ring (they die in the prepass, scores first exist after it), x1T streaming
chunks and all x2T chunks share one 24-slot ring whose reuse order matches
first-use order, and mrow/srow/rrow share one ring.

Engine placement: PSUM->SBUF score copies on ScalarE, the (scores - max)
subtract on GpSimdE, max-chase/reduces/normalize on VectorE, so no engine
queue ever gates the PE at a phase boundary.

Host side: transposes x1/x2/U/fc_w per-core (fp32 DMA transpose does not
exist on TRN2) and transposes the [F,S] per-core output back to [S,F].
"""

import os
import sys
from contextlib import ExitStack

import numpy as np

for _p in ("/opt/trn_rl_repo", os.path.expanduser("~/.axon_site/_ro/trn_rl_repo")):
    if os.path.isdir(_p) and _p not in sys.path:
        sys.path.insert(0, _p)

import concourse.bass as bass
import concourse.mybir as mybir
import concourse.tile as tile
from concourse import bacc

B = 8
S = 2048          # sequence length (both s and t')
D = 1024          # d_model
F = 512           # fc output dim
P = 128
SB = 512          # s superblock (moving free dim of every matmul)
NSB = S // SB     # 4
DC = D // P       # 8 contraction chunks of d / e
TC = S // P       # 16 t' tiles
FPG = F // P      # 4 output row-tiles
FP32 = mybir.dt.float32
FP32R = mybir.dt.float32r
BF16 = mybir.dt.bfloat16
AF = mybir.ActivationFunctionType
ALU = mybir.AluOpType
AX = mybir.AxisListType


def build_nc():
    nc = bacc.Bacc(
        "TRN2",
        target_bir_lowering=False,
        debug=False,
        enable_asserts=False,
    )

    x1t_d = nc.dram_tensor("x1t", [D, S], FP32R, kind="ExternalInput")
    x2t_d = nc.dram_tensor("x2t", [D, S], FP32R, kind="ExternalInput")
    ut_d = nc.dram_tensor("ut", [D, D], FP32R, kind="ExternalInput")
    fcwt_d = nc.dram_tensor("fcwt", [D, F], FP32R, kind="ExternalInput")
    bias_d = nc.dram_tensor("bias", [D, 1], FP32R, kind="ExternalInput")
    fcb_d = nc.dram_tensor("fcb", [F, 1], FP32, kind="ExternalInput")
    outt_d = nc.dram_tensor("outt", [F, S], FP32, kind="ExternalOutput")

    with tile.TileContext(nc) as tc, ExitStack() as ctx:
        # ---------- pools ----------
        p_y1t = ctx.enter_context(tc.tile_pool(name="y1ts", bufs=NSB * DC))
        p_stream = ctx.enter_context(tc.tile_pool(name="stream", bufs=24))
        p_sc = ctx.enter_context(tc.tile_pool(name="scores", bufs=TC))
        p_x1fc = ctx.enter_context(tc.tile_pool(name="x1fcs", bufs=TC))
        p_fcw = ctx.enter_context(tc.tile_pool(name="fcwres", bufs=DC))
        p_kb = ctx.enter_context(tc.tile_pool(name="kbcols", bufs=TC))
        p_bc = ctx.enter_context(tc.tile_pool(name="biascols", bufs=DC))
        p_fcb = ctx.enter_context(tc.tile_pool(name="fcbcols", bufs=FPG))
        p_ones = ctx.enter_context(tc.tile_pool(name="ones", bufs=1))
        p_pb = ctx.enter_context(tc.tile_pool(name="pbf", bufs=6))
        p_aux = ctx.enter_context(tc.tile_pool(name="aux", bufs=1))
        p_row = ctx.enter_context(tc.tile_pool(name="rows", bufs=1))
        p_mcol = ctx.enter_context(tc.tile_pool(name="mcols", bufs=SB // P))
        p_out = ctx.enter_context(tc.tile_pool(name="outs", bufs=2))
        # PSUM: 4 banks for the output accumulators, 1 for the rowsum, 3
        # general-purpose (MM2/transposes/broadcasts/prepass) = 8 banks.
        p_pso = ctx.enter_context(tc.tile_pool(name="pso", bufs=FPG, space="PSUM"))
        p_pssum = ctx.enter_context(tc.tile_pool(name="pssum", bufs=1, space="PSUM"))
        p_psg = ctx.enter_context(tc.tile_pool(name="psg", bufs=3, space="PSUM"))

        # ---------- constants (no DMA dependency) ----------
        identity = p_ones.tile([P, P], FP32, name="ident", tag="ident")
        nc.gpsimd.memset(identity[:], 0.0)
        nc.gpsimd.affine_select(
            out=identity[:], in_=identity[:], compare_op=ALU.not_equal,
            fill=1.0, base=0, pattern=[[-1, P]], channel_multiplier=1,
        )
        ones_row = p_ones.tile([1, P], FP32R, name="ones_row", tag="ones_row")
        nc.scalar.activation(ones_row[:], identity[0:1, :], AF.Identity, bias=1.0, scale=0.0)
        ones_col = p_ones.tile([P, 1], BF16, name="ones_col", tag="ones_col")
        nc.scalar.activation(ones_col[:], identity[:, 0:1], AF.Identity, bias=1.0, scale=0.0)

        # ---------- input DMAs ----------
        # Emission order is DMA queue order AND stream-ring slot order: the
        # ring reuses slots in first-use order (x1T tg chunks die tg-by-tg in
        # the prepass, freeing slots for later chunks / x2T superblocks).
        bias_cols = []
        for i in range(DC):
            b_t = p_bc.tile([P, 1], FP32R, name=f"bc{i}", tag="bc")
            nc.sync.dma_start(b_t[:], bias_d[i * P : (i + 1) * P, :])
            bias_cols.append(b_t)
        chunks = [[None] * DC for _ in range(NSB)]

        def load_chunks(tg):
            # split each chunk across two DMA queues (partition halves) —
            # a single 256KB transfer is queue-latency-bound at startup
            for ec in range(DC):
                c = p_stream.tile([P, SB], FP32R, name=f"x1tc_{tg}_{ec}", tag="st")
                h = P // 2
                for hf in range(2):
                    nc.sync.dma_start(
                        c[hf * h : (hf + 1) * h, :],
                        x1t_d[
                            ec * P + hf * h : ec * P + (hf + 1) * h,
                            tg * SB : (tg + 1) * SB,
                        ],
                    )
                chunks[tg][ec] = c

        def emit_x2t(sb):
            s0 = sb * SB
            x2t_tiles = []
            for dc in range(DC):
                x2_t = p_stream.tile([P, SB], FP32R, name=f"x2t_{sb}_{dc}", tag="st")
                nc.sync.dma_start(x2_t[:], x2t_d[dc * P : (dc + 1) * P, s0 : s0 + SB])
                x2t_tiles.append(x2_t)
            return x2t_tiles

        load_chunks(0)
        fcw_tiles = []
        for i in range(DC):
            f_t = p_fcw.tile([P, F], FP32R, name=f"fcw{i}", tag="fcw")
            nc.sync.dma_start(f_t[:], fcwt_d[i * P : (i + 1) * P, :])
            fcw_tiles.append(f_t)
        # U^T tiles borrow the score ring: they die in the prepass, before
        # the first score tile exists.
        ut_tiles = [[None, None] for _ in range(DC)]
        for ec in range(DC):
            for h in range(2):
                t = p_sc.tile([P, SB], FP32R, name=f"ut{ec}_{h}", tag="sc")
                nc.sync.dma_start(
                    t[:], ut_d[ec * P : (ec + 1) * P, h * SB : (h + 1) * SB]
                )
                ut_tiles[ec][h] = t
        load_chunks(1)
        load_chunks(2)
        load_chunks(3)
        x2t_cur = emit_x2t(0)
        fcb_cols = []
        for i in range(FPG):
            c_t = p_fcb.tile([P, 1], FP32, name=f"fcb{i}", tag="fcb")
            nc.sync.dma_start(c_t[:], fcb_d[i * P : (i + 1) * P, :])
            fcb_cols.append(c_t)

        # ---------- prepass: kb = x1 @ bias, x1fc = x1 @ fc_w^T,
        #                     y1T = U @ x1^T ----------
        kb_cols = [None] * TC
        x1fc_tiles = [None] * TC
        y1t_tiles = [[None] * NSB for _ in range(DC)]
        for tg in range(NSB):
            ps_kb = p_pssum.tile([1, SB], FP32, name=f"pskb{tg}", tag="pssum")
            for ec in range(DC):
                nc.tensor.matmul(
                    ps_kb[:], bias_cols[ec][:], chunks[tg][ec][:],
                    start=(ec == 0), stop=(ec == DC - 1),
                )
            kb_r = p_row.tile([1, SB], FP32R, name=f"kbrow{tg}", tag="kbrow")
            nc.vector.tensor_copy(kb_r[:], ps_kb[:])
            for sub in range(4):
                ti = tg * 4 + sub
                ps_x = p_psg.tile([P, F], FP32, name=f"psx{ti}", tag="psg")
                for ec in range(DC):
                    nc.tensor.matmul(
                        ps_x[:],
                        chunks[tg][ec][:, sub * P : (sub + 1) * P],
                        fcw_tiles[ec][:],
                        start=(ec == 0), stop=(ec == DC - 1),
                    )
                xf = p_x1fc.tile([P, F], BF16, name=f"x1fc{ti}", tag="x1fc")
                nc.vector.tensor_copy(xf[:], ps_x[:])
                x1fc_tiles[ti] = xf
                ps_c = p_psg.tile([P, 4], FP32, name=f"pskc{ti}", tag="psg")
                nc.tensor.matmul(
                    ps_c[:], kb_r[0:1, sub * P : (sub + 1) * P],
                    ones_row[0:1, 0:4], start=True, stop=True,
                )
                kb_c = p_kb.tile([P, 1], FP32, name=f"kb{ti}", tag="kb")
                nc.vector.tensor_copy(kb_c[:], ps_c[:, 0:1])
                kb_cols[ti] = kb_c
            for dc in range(DC):
                ps_y = p_psg.tile([P, SB], FP32, name=f"psy{tg}_{dc}", tag="psg")
                for ec in range(DC):
                    nc.tensor.matmul(
                        ps_y[:],
                        ut_tiles[ec][dc // 4][:, (dc % 4) * P : (dc % 4 + 1) * P],
                        chunks[tg][ec][:],
                        start=(ec == 0), stop=(ec == DC - 1),
                    )
                y_t = p_y1t.tile([P, SB], FP32R, name=f"y1t{dc}_{tg}", tag="y1t")
                nc.vector.tensor_copy(y_t[:], ps_y[:])
                y1t_tiles[dc][tg] = y_t

        # ---------- steady loop ----------
        # MM2 tile emitter; the first two tiles of each superblock are
        # emitted during the previous superblock's tail so the PE has work
        # while VectorE runs the recip chain.
        sc_all = [[None] * TC for _ in range(NSB)]
        maxaccs = [None] * NSB

        def mm2_tile(sb, ti, x2t_tiles):
            tg, sub = ti // 4, ti % 4
            ps_s = p_psg.tile([P, SB], FP32, name=f"pss{sb}_{ti}", tag="psg")
            for dc in range(DC):
                nc.tensor.matmul(
                    ps_s[:],
                    y1t_tiles[dc][tg][:, sub * P : (sub + 1) * P],
                    x2t_tiles[dc][:],
                    start=(dc == 0), stop=(dc == DC - 1),
                )
            s_t = p_sc.tile([P, SB], FP32, name=f"sc{sb}_{ti}", tag="sc")
            nc.scalar.copy(s_t[:], ps_s[:])
            if ti == 0:
                maxaccs[sb] = p_aux.tile(
                    [P, SB], FP32, name=f"maxacc{sb}", tag="maxacc"
                )
                nc.gpsimd.tensor_copy(maxaccs[sb][:], s_t[:])
            else:
                nc.vector.tensor_max(maxaccs[sb][:], maxaccs[sb][:], s_t[:])
            sc_all[sb][ti] = s_t

        for sb in range(NSB):
            s0 = sb * SB
            x2t_tiles = x2t_cur
            if sb + 1 < NSB:
                x2t_cur = emit_x2t(sb + 1)

            # ---- MM2: scoresT tiles + running elementwise max ----
            for ti in range(0 if sb == 0 else 2, TC):
                mm2_tile(sb, ti, x2t_tiles)
            sc_tiles = sc_all[sb]
            maxacc = maxaccs[sb]

            # ---- per-s max over partitions: transpose + free reduce ----
            mrow = p_row.tile([1, SB], FP32R, name=f"mrow{sb}", tag="rowsc")
            mcols = []
            for blk in range(SB // P):
                ps_tr = p_psg.tile([P, P], FP32, name=f"ptr{sb}_{blk}", tag="psg")
                nc.tensor.transpose(
                    ps_tr[:], maxacc[:, blk * P : (blk + 1) * P], identity[:]
                )
                mcol = p_mcol.tile([P, 1], FP32, name=f"mcol{sb}_{blk}", tag="mcol")
                nc.vector.reduce_max(mcol[:], ps_tr[:], axis=AX.X)
                mcols.append(mcol)
            for blk in range(SB // P):
                ps_rr = p_psg.tile([1, P], FP32, name=f"prr{sb}_{blk}", tag="psg")
                nc.tensor.transpose(ps_rr[:], mcols[blk][:], identity[:])
                nc.vector.tensor_copy(mrow[:, blk * P : (blk + 1) * P], ps_rr[:])
            ps_mb = p_psg.tile([P, SB], FP32, name=f"pmb{sb}", tag="psg")
            nc.tensor.matmul(ps_mb[:], ones_row[:], mrow[:], start=True, stop=True)
            maxb = p_aux.tile([P, SB], FP32, name=f"maxb{sb}", tag="maxb")
            nc.scalar.copy(maxb[:], ps_mb[:])

            # ---- exp phase, chased by the PE: rowsum + output matmuls ----
            ps_sum = p_pssum.tile([1, SB], FP32, name=f"pssm{sb}", tag="pssum")
            ps_o = [
                p_pso.tile([P, SB], FP32, name=f"pso{sb}_{ft}", tag="pso")
                for ft in range(FPG)
            ]
            for ti in range(TC):
                nc.gpsimd.tensor_sub(sc_tiles[ti][:], sc_tiles[ti][:], maxb[:])
                p_t = p_pb.tile([P, SB], BF16, name=f"pb{sb}_{ti}", tag="pb")
                nc.scalar.activation(
                    p_t[:], sc_tiles[ti][:], AF.Exp, bias=kb_cols[ti][:], scale=1.0
                )
                nc.tensor.matmul(
                    ps_sum[:], ones_col[:], p_t[:],
                    start=(ti == 0), stop=(ti == TC - 1),
                )
                for ft in range(FPG):
                    nc.tensor.matmul(
                        ps_o[ft][:],
                        x1fc_tiles[ti][:, ft * P : (ft + 1) * P],
                        p_t[:],
                        start=(ti == 0), stop=(ti == TC - 1),
                    )

            # next superblock's first MM2 tiles backfill the recip handoff
            if sb + 1 < NSB:
                for ti in range(2):
                    mm2_tile(sb + 1, ti, x2t_cur)

            # ---- recip + broadcast ----
            srow = p_row.tile([1, SB], FP32, name=f"srow{sb}", tag="rowsc")
            nc.vector.tensor_copy(srow[:], ps_sum[:])
            rrow = p_row.tile([1, SB], FP32R, name=f"rrow{sb}", tag="kbrow")
            with nc.allow_low_precision(reason="recip feeds fp32r matmul; fp22 ok"):
                nc.vector.reciprocal(rrow[:], srow[:])
            ps_rb = p_psg.tile([P, SB], FP32, name=f"prb{sb}", tag="psg")
            nc.tensor.matmul(ps_rb[:], ones_row[:], rrow[:], start=True, stop=True)
            recipb = p_aux.tile([P, SB], FP32, name=f"recipb{sb}", tag="recipb")
            nc.vector.tensor_copy(recipb[:], ps_rb[:])

            # ---- normalize + bias + relu + store ----
            for ft in range(FPG):
                tmp = p_out.tile([P, SB], FP32, name=f"tmp{sb}_{ft}", tag="tmp")
                nc.vector.tensor_mul(tmp[:], ps_o[ft][:], recipb[:])
                o_out = p_out.tile([P, SB], FP32, name=f"oo{sb}_{ft}", tag="oo")
                nc.scalar.activation(
                    o_out[:], tmp[:], AF.Relu, bias=fcb_cols[ft][:], scale=1.0
                )
                nc.sync.dma_start(outt_d[ft * P : (ft + 1) * P, s0 : s0 + SB], o_out[:])

    nc.compile()
    return nc


_NC_CACHE = None


def _get_nc():
    global _NC_CACHE
    if _NC_CACHE is None:
        _NC_CACHE = build_nc()
    return _NC_CACHE


def make_in_maps(x1, x2, U, bias, fc_w, fc_b):
    x1 = np.ascontiguousarray(np.asarray(x1, dtype=np.float32))
    x2 = np.ascontiguousarray(np.asarray(x2, dtype=np.float32))
    U = np.asarray(U, dtype=np.float32)
    bias = np.asarray(bias, dtype=np.float32).reshape(D, 1)
    fc_w = np.asarray(fc_w, dtype=np.float32)
    fc_b = np.asarray(fc_b, dtype=np.float32).reshape(F, 1)
    ut = np.ascontiguousarray(U.T)
    fcwt = np.ascontiguousarray(fc_w.T)
    in_maps = []
    for b in range(B):
        in_maps.append(
            {
                "x1t": np.ascontiguousarray(x1[b].T),
                "x2t": np.ascontiguousarray(x2[b].T),
                "ut": ut,
                "fcwt": fcwt,
                "bias": bias,
                "fcb": fc_b,
            }
        )
    return in_maps


def kernel(x1, x2, U, bias, fc_w, fc_b):
    from concourse.bass_utils import run_bass_kernel_spmd

    nc = _get_nc()
    in_maps = make_in_maps(x1, x2, U, bias, fc_w, fc_b)
    res = run_bass_kernel_spmd(nc, in_maps, core_ids=list(range(B)))
    out = np.stack([np.ascontiguousarray(r["outt"].T) for r in res.results])
    return out.astype(np.float32)


# revision 21
# speedup vs baseline: 1.1914x; 1.0227x over previous
"""BiaffineAttn Trainium2 kernel.

Math (per batch b):
    t    = x2 @ U                      [S, D]
    attn = t @ x1^T + (x1 @ bias)[None, :]
    p    = softmax(attn, axis=-1)
    out  = relu((p @ x1) @ fc_w^T + fc_b)    [S, F]

Sharding: data-parallel over batch B=8, one batch per NeuronCore.

Two algebraic restructures vs the naive form:
  * (p @ x1) @ fc_w^T = p @ (x1 @ fc_w^T): a one-time prepass
    x1fc = x1 @ fc_w^T [S, F] (bf16) replaces the per-superblock
    S*S*D + S*D*F output matmuls with S*S*F — 40% fewer output-side MACs.
  * (x2 @ U) @ x1^T = x2 @ (U @ x1^T): a one-time prepass
    y1T = U @ x1^T [D, S] (fp32r) absorbs the whole x2-side projection into
    startup — same FLOPs, but the steady loop loses the per-superblock tT
    matmul phase and its PSUM/copy traffic entirely, which keeps the tensor
    engine in long uninterrupted matmul runs (the HAM clock gate throttles
    the PE array to half clock after ~3.4us of low activity).

Per-core pipeline (attention in TRANSPOSED orientation so the softmax key
dimension t' lands on SBUF partitions; N=512 moving chunks):
    scoresT = attn^T [t', s]        fp32r y1T 128x128 stationaries, fp32r x2T
    pT      = exp(scoresT - rowmax_bcast + kb)  (kb = x1 @ bias folded into
                                    exp's per-partition bias operand)
    rowsum  = ones_col^T @ pT       PE K-accumulated 1-row matmul (no DVE
                                    reduce pipeline)
    outT    = relu(recip * (x1fc^T @ pT) + fcb)   stationary x1fc tiles
  rowmax: elementwise-max chase over the 16 t'-tiles on VectorE, then a
  128-partition reduce via PE transposes + ones-matmul broadcast.

The score path stays fp32r (fp22) end-to-end: with scores ~N(0, 32^2) the
softmax is highly peaked and any bf16 rounding upstream (~0.2 absolute on
scores) turns into ~20% errors on near-tie attention weights, blowing the
max-abs error metric.  bf16 is confined to the post-exp path (p, x1fc),
where errors average out over the 2048-key contraction.

SBUF packing (the y1T residency costs 8MB): the U^T tiles borrow the score
ring (they die in the prepass, scores first exist after it), x1T streaming
chunks and all x2T chunks share one 24-slot ring whose reuse order matches
first-use order, and mrow/srow/rrow share one ring.

Engine placement: PSUM->SBUF score copies on ScalarE, the (scores - max)
subtract on GpSimdE, max-chase/reduces/normalize on VectorE, so no engine
queue ever gates the PE at a phase boundary.

Host side: transposes x1/x2/U/fc_w per-core (fp32 DMA transpose does not
exist on TRN2) and transposes the [F,S] per-core output back to [S,F].
"""

import os
import sys
from contextlib import ExitStack

import numpy as np

for _p in ("/opt/trn_rl_repo", os.path.expanduser("~/.axon_site/_ro/trn_rl_repo")):
    if os.path.isdir(_p) and _p not in sys.path:
        sys.path.insert(0, _p)

import concourse.bass as bass
import concourse.mybir as mybir
import concourse.tile as tile
from concourse import bacc

B = 8
S = 2048          # sequence length (both s and t')
D = 1024          # d_model
F = 512           # fc output dim
P = 128
SB = 512          # s superblock (moving free dim of every matmul)
NSB = S // SB     # 4
DC = D // P       # 8 contraction chunks of d / e
TC = S // P       # 16 t' tiles
FPG = F // P      # 4 output row-tiles
FP32 = mybir.dt.float32
FP32R = mybir.dt.float32r
BF16 = mybir.dt.bfloat16
AF = mybir.ActivationFunctionType
ALU = mybir.AluOpType
AX = mybir.AxisListType


def build_nc():
    nc = bacc.Bacc(
        "TRN2",
        target_bir_lowering=False,
        debug=False,
        enable_asserts=False,
    )

    x1t_d = nc.dram_tensor("x1t", [D, S], FP32R, kind="ExternalInput")
    x2t_d = nc.dram_tensor("x2t", [D, S], FP32R, kind="ExternalInput")
    ut_d = nc.dram_tensor("ut", [D, D], FP32R, kind="ExternalInput")
    fcwt_d = nc.dram_tensor("fcwt", [D, F], FP32R, kind="ExternalInput")
    bias_d = nc.dram_tensor("bias", [D, 1], FP32R, kind="ExternalInput")
    fcb_d = nc.dram_tensor("fcb", [F, 1], FP32, kind="ExternalInput")
    outt_d = nc.dram_tensor("outt", [F, S], FP32, kind="ExternalOutput")

    with tile.TileContext(nc) as tc, ExitStack() as ctx:
        # ---------- pools ----------
        p_y1t = ctx.enter_context(tc.tile_pool(name="y1ts", bufs=NSB * DC))
        p_stream = ctx.enter_context(tc.tile_pool(name="stream", bufs=24))
        p_sc = ctx.enter_context(tc.tile_pool(name="scores", bufs=TC))
        p_x1fc = ctx.enter_context(tc.tile_pool(name="x1fcs", bufs=TC))
        p_fcw = ctx.enter_context(tc.tile_pool(name="fcwres", bufs=DC))
        p_kb = ctx.enter_context(tc.tile_pool(name="kbcols", bufs=TC))
        p_bc = ctx.enter_context(tc.tile_pool(name="biascols", bufs=DC))
        p_fcb = ctx.enter_context(tc.tile_pool(name="fcbcols", bufs=FPG))
        p_ones = ctx.enter_context(tc.tile_pool(name="ones", bufs=1))
        p_pb = ctx.enter_context(tc.tile_pool(name="pbf", bufs=6))
        p_aux = ctx.enter_context(tc.tile_pool(name="aux", bufs=1))
        p_row = ctx.enter_context(tc.tile_pool(name="rows", bufs=1))
        p_mcol = ctx.enter_context(tc.tile_pool(name="mcols", bufs=SB // P))
        p_out = ctx.enter_context(tc.tile_pool(name="outs", bufs=2))
        # PSUM: 4 banks for the output accumulators, 1 for the rowsum, 3
        # general-purpose (MM2/transposes/broadcasts/prepass) = 8 banks.
        p_pso = ctx.enter_context(tc.tile_pool(name="pso", bufs=FPG, space="PSUM"))
        p_pssum = ctx.enter_context(tc.tile_pool(name="pssum", bufs=1, space="PSUM"))
        p_psg = ctx.enter_context(tc.tile_pool(name="psg", bufs=3, space="PSUM"))

        # ---------- constants (no DMA dependency) ----------
        identity = p_ones.tile([P, P], FP32, name="ident", tag="ident")
        nc.gpsimd.memset(identity[:], 0.0)
        nc.gpsimd.affine_select(
            out=identity[:], in_=identity[:], compare_op=ALU.not_equal,
            fill=1.0, base=0, pattern=[[-1, P]], channel_multiplier=1,
        )
        ones_row = p_ones.tile([1, P], FP32R, name="ones_row", tag="ones_row")
        nc.scalar.activation(ones_row[:], identity[0:1, :], AF.Identity, bias=1.0, scale=0.0)
        ones_col = p_ones.tile([P, 1], BF16, name="ones_col", tag="ones_col")
        nc.scalar.activation(ones_col[:], identity[:, 0:1], AF.Identity, bias=1.0, scale=0.0)

        # ---------- input DMAs ----------
        # Emission order is DMA queue order AND stream-ring slot order: the
        # ring reuses slots in first-use order (x1T tg chunks die tg-by-tg in
        # the prepass, freeing slots for later chunks / x2T superblocks).
        bias_cols = []
        for i in range(DC):
            b_t = p_bc.tile([P, 1], FP32R, name=f"bc{i}", tag="bc")
            nc.sync.dma_start(b_t[:], bias_d[i * P : (i + 1) * P, :])
            bias_cols.append(b_t)
        chunks = [[None] * DC for _ in range(NSB)]

        def load_chunks(tg):
            # split each chunk across two DMA queues (partition halves) —
            # a single 256KB transfer is queue-latency-bound at startup
            for ec in range(DC):
                c = p_stream.tile([P, SB], FP32R, name=f"x1tc_{tg}_{ec}", tag="st")
                h = P // 2
                for hf in range(2):
                    nc.sync.dma_start(
                        c[hf * h : (hf + 1) * h, :],
                        x1t_d[
                            ec * P + hf * h : ec * P + (hf + 1) * h,
                            tg * SB : (tg + 1) * SB,
                        ],
                    )
                chunks[tg][ec] = c

        def emit_x2t(sb):
            s0 = sb * SB
            x2t_tiles = []
            for dc in range(DC):
                x2_t = p_stream.tile([P, SB], FP32R, name=f"x2t_{sb}_{dc}", tag="st")
                nc.sync.dma_start(x2_t[:], x2t_d[dc * P : (dc + 1) * P, s0 : s0 + SB])
                x2t_tiles.append(x2_t)
            return x2t_tiles

        load_chunks(0)
        fcw_tiles = []
        for i in range(DC):
            f_t = p_fcw.tile([P, F], FP32R, name=f"fcw{i}", tag="fcw")
            nc.sync.dma_start(f_t[:], fcwt_d[i * P : (i + 1) * P, :])
            fcw_tiles.append(f_t)
        # U^T tiles borrow the score ring: they die in the prepass, before
        # the first score tile exists.
        ut_tiles = [[None, None] for _ in range(DC)]
        for ec in range(DC):
            for h in range(2):
                t = p_sc.tile([P, SB], FP32R, name=f"ut{ec}_{h}", tag="sc")
                nc.sync.dma_start(
                    t[:], ut_d[ec * P : (ec + 1) * P, h * SB : (h + 1) * SB]
                )
                ut_tiles[ec][h] = t
        load_chunks(1)
        load_chunks(2)
        load_chunks(3)
        x2t_cur = emit_x2t(0)
        fcb_cols = []
        for i in range(FPG):
            c_t = p_fcb.tile([P, 1], FP32, name=f"fcb{i}", tag="fcb")
            nc.sync.dma_start(c_t[:], fcb_d[i * P : (i + 1) * P, :])
            fcb_cols.append(c_t)

        # ---------- prepass: kb = x1 @ bias, x1fc = x1 @ fc_w^T,
        #                     y1T = U @ x1^T ----------
        kb_cols = [None] * TC
        x1fc_tiles = [None] * TC
        y1t_tiles = [[None] * NSB for _ in range(DC)]
        for tg in range(NSB):
            ps_kb = p_pssum.tile([1, SB], FP32, name=f"pskb{tg}", tag="pssum")
            for ec in range(DC):
                nc.tensor.matmul(
                    ps_kb[:], bias_cols[ec][:], chunks[tg][ec][:],
                    start=(ec == 0), stop=(ec == DC - 1),
                )
            kb_r = p_row.tile([1, SB], FP32R, name=f"kbrow{tg}", tag="kbrow")
            nc.vector.tensor_copy(kb_r[:], ps_kb[:])
            for sub in range(4):
                ti = tg * 4 + sub
                ps_x = p_psg.tile([P, F], FP32, name=f"psx{ti}", tag="psg")
                for ec in range(DC):
                    nc.tensor.matmul(
                        ps_x[:],
                        chunks[tg][ec][:, sub * P : (sub + 1) * P],
                        fcw_tiles[ec][:],
                        start=(ec == 0), stop=(ec == DC - 1),
                    )
                xf = p_x1fc.tile([P, F], BF16, name=f"x1fc{ti}", tag="x1fc")
                nc.vector.tensor_copy(xf[:], ps_x[:])
                x1fc_tiles[ti] = xf
                ps_c = p_psg.tile([P, 4], FP32, name=f"pskc{ti}", tag="psg")
                nc.tensor.matmul(
                    ps_c[:], kb_r[0:1, sub * P : (sub + 1) * P],
                    ones_row[0:1, 0:4], start=True, stop=True,
                )
                kb_c = p_kb.tile([P, 1], FP32, name=f"kb{ti}", tag="kb")
                nc.vector.tensor_copy(kb_c[:], ps_c[:, 0:1])
                kb_cols[ti] = kb_c
            for dc in range(DC):
                ps_y = p_psg.tile([P, SB], FP32, name=f"psy{tg}_{dc}", tag="psg")
                for ec in range(DC):
                    nc.tensor.matmul(
                        ps_y[:],
                        ut_tiles[ec][dc // 4][:, (dc % 4) * P : (dc % 4 + 1) * P],
                        chunks[tg][ec][:],
                        start=(ec == 0), stop=(ec == DC - 1),
                    )
                y_t = p_y1t.tile([P, SB], FP32R, name=f"y1t{dc}_{tg}", tag="y1t")
                nc.vector.tensor_copy(y_t[:], ps_y[:])
                y1t_tiles[dc][tg] = y_t

        # ---------- steady loop ----------
        # MM2 tile emitter; the first two tiles of each superblock are
        # emitted during the previous superblock's tail so the PE has work
        # while VectorE runs the recip chain.
        sc_all = [[None] * TC for _ in range(NSB)]
        maxaccs = [None] * NSB

        def mm2_tile(sb, ti, x2t_tiles):
            tg, sub = ti // 4, ti % 4
            ps_s = p_psg.tile([P, SB], FP32, name=f"pss{sb}_{ti}", tag="psg")
            for dc in range(DC):
                nc.tensor.matmul(
                    ps_s[:],
                    y1t_tiles[dc][tg][:, sub * P : (sub + 1) * P],
                    x2t_tiles[dc][:],
                    start=(dc == 0), stop=(dc == DC - 1),
                )
            s_t = p_sc.tile([P, SB], FP32, name=f"sc{sb}_{ti}", tag="sc")
            nc.scalar.copy(s_t[:], ps_s[:])
            if ti == 0:
                maxaccs[sb] = p_aux.tile(
                    [P, SB], FP32, name=f"maxacc{sb}", tag="maxacc"
                )
                nc.gpsimd.tensor_copy(maxaccs[sb][:], s_t[:])
            else:
                nc.vector.tensor_max(maxaccs[sb][:], maxaccs[sb][:], s_t[:])
            sc_all[sb][ti] = s_t

        for sb in range(NSB):
            s0 = sb * SB
            x2t_tiles = x2t_cur
            if sb + 1 < NSB:
                x2t_cur = emit_x2t(sb + 1)

            # ---- MM2: scoresT tiles + running elementwise max ----
            for ti in range(0 if sb == 0 else 2, TC):
                mm2_tile(sb, ti, x2t_tiles)
            sc_tiles = sc_all[sb]
            maxacc = maxaccs[sb]

            # ---- per-s max over partitions: transpose + free reduce ----
            mrow = p_row.tile([1, SB], FP32R, name=f"mrow{sb}", tag="rowsc")
            mcols = []
            for blk in range(SB // P):
                ps_tr = p_psg.tile([P, P], FP32, name=f"ptr{sb}_{blk}", tag="psg")
                nc.tensor.transpose(
                    ps_tr[:], maxacc[:, blk * P : (blk + 1) * P], identity[:]
                )
                mcol = p_mcol.tile([P, 1], FP32, name=f"mcol{sb}_{blk}", tag="mcol")
                nc.vector.reduce_max(mcol[:], ps_tr[:], axis=AX.X)
                mcols.append(mcol)
            for blk in range(SB // P):
                ps_rr = p_psg.tile([1, P], FP32, name=f"prr{sb}_{blk}", tag="psg")
                nc.tensor.transpose(ps_rr[:], mcols[blk][:], identity[:])
                nc.vector.tensor_copy(mrow[:, blk * P : (blk + 1) * P], ps_rr[:])
            ps_mb = p_psg.tile([P, SB], FP32, name=f"pmb{sb}", tag="psg")
            nc.tensor.matmul(ps_mb[:], ones_row[:], mrow[:], start=True, stop=True)
            maxb = p_aux.tile([P, SB], FP32, name=f"maxb{sb}", tag="maxb")
            nc.scalar.copy(maxb[:], ps_mb[:])

            # ---- exp phase, chased by the PE: rowsum + output matmuls ----
            ps_sum = p_pssum.tile([1, SB], FP32, name=f"pssm{sb}", tag="pssum")
            ps_o = [
                p_pso.tile([P, SB], FP32, name=f"pso{sb}_{ft}", tag="pso")
                for ft in range(FPG)
            ]
            for ti in range(TC):
                nc.gpsimd.tensor_sub(sc_tiles[ti][:], sc_tiles[ti][:], maxb[:])
                p_t = p_pb.tile([P, SB], BF16, name=f"pb{sb}_{ti}", tag="pb")
                nc.scalar.activation(
                    p_t[:], sc_tiles[ti][:], AF.Exp, bias=kb_cols[ti][:], scale=1.0
                )
                nc.tensor.matmul(
                    ps_sum[:], ones_col[:], p_t[:],
                    start=(ti == 0), stop=(ti == TC - 1),
                )
                for ft in range(FPG):
                    nc.tensor.matmul(
                        ps_o[ft][:],
                        x1fc_tiles[ti][:, ft * P : (ft + 1) * P],
                        p_t[:],
                        start=(ti == 0), stop=(ti == TC - 1),
                    )

            # next superblock's first MM2 tiles backfill the recip handoff
            if sb + 1 < NSB:
                for ti in range(2):
                    mm2_tile(sb + 1, ti, x2t_cur)

            # ---- recip + broadcast ----
            srow = p_row.tile([1, SB], FP32, name=f"srow{sb}", tag="rowsc")
            nc.vector.tensor_copy(srow[:], ps_sum[:])
            rrow = p_row.tile([1, SB], FP32R, name=f"rrow{sb}", tag="kbrow")
            with nc.allow_low_precision(reason="recip feeds fp32r matmul; fp22 ok"):
                nc.vector.reciprocal(rrow[:], srow[:])
            ps_rb = p_psg.tile([P, SB], FP32, name=f"prb{sb}", tag="psg")
            nc.tensor.matmul(ps_rb[:], ones_row[:], rrow[:], start=True, stop=True)
            recipb = p_aux.tile([P, SB], FP32, name=f"recipb{sb}", tag="recipb")
            nc.vector.tensor_copy(recipb[:], ps_rb[:])

            # ---- normalize + bias + relu + store ----
            for ft in range(FPG):
                tmp = p_out.tile([P, SB], FP32, name=f"tmp{sb}_{ft}", tag="tmp")
                nc.vector.tensor_mul(tmp[:], ps_o[ft][:], recipb[:])
                o_out = p_out.tile([P, SB], FP32, name=f"oo{sb}_{ft}", tag="oo")
                nc.scalar.activation(
                    o_out[:], tmp[:], AF.Relu, bias=fcb_cols[ft][:], scale=1.0
                )
                nc.sync.dma_start(outt_d[ft * P : (ft + 1) * P, s0 : s0 + SB], o_out[:])

    nc.compile()
    return nc


_NC_CACHE = None


def _get_nc():
    global _NC_CACHE
    if _NC_CACHE is None:
        _NC_CACHE = build_nc()
    return _NC_CACHE


def make_in_maps(x1, x2, U, bias, fc_w, fc_b):
    x1 = np.ascontiguousarray(np.asarray(x1, dtype=np.float32))
    x2 = np.ascontiguousarray(np.asarray(x2, dtype=np.float32))
    U = np.asarray(U, dtype=np.float32)
    bias = np.asarray(bias, dtype=np.float32).reshape(D, 1)
    fc_w = np.asarray(fc_w, dtype=np.float32)
    fc_b = np.asarray(fc_b, dtype=np.float32).reshape(F, 1)
    ut = np.ascontiguousarray(U.T)
    fcwt = np.ascontiguousarray(fc_w.T)
    in_maps = []
    for b in range(B):
        in_maps.append(
            {
                "x1t": np.ascontiguousarray(x1[b].T),
                "x2t": np.ascontiguousarray(x2[b].T),
                "ut": ut,
                "fcwt": fcwt,
                "bias": bias,
                "fcb": fc_b,
            }
        )
    return in_maps


def kernel(x1, x2, U, bias, fc_w, fc_b):
    from concourse.bass_utils import run_bass_kernel_spmd

    nc = _get_nc()
    in_maps = make_in_maps(x1, x2, U, bias, fc_w, fc_b)
    res = run_bass_kernel_spmd(nc, in_maps, core_ids=list(range(B)))
    out = np.stack([np.ascontiguousarray(r["outt"].T) for r in res.results])
    return out.astype(np.float32)
